# revision 18
# baseline (speedup 1.0000x reference)
"""Trainium2 Bass kernel: caching self multi-headed attention (decode step).

Problem: B=32, QLEN=1, DM=1024, H=16, DK=64, TCACHE=4096, fp32 in/out.
  out = MHA(q; KV cache) with QKV projections, cache append, softmax, out-proj.

Sharding (8 NeuronCores): tensor-parallel over heads. Core c owns heads
[2c, 2c+1]: column-parallel wq/wk/wv (128 output dims per core), KV cache
shards on the head dim, row-parallel wo giving a partial [32, 1024] output
per core; the host sums the 8 partials.

Design (final), from HW traces (v1 fp32: DMA 88%@314 GB/s, PE 91%; bf16
merged-DMA: stream runs ~425 GB/s; measured exec 206-208 us matches the
chip-wide HBM floor: 8 cores x 68.5 MB / ~2.95 TB/s + ~17 us preamble/tail;
a pure-DMA variant with no compute measures the same, so compute is fully
hidden):
  * KV cache bf16 on host -> 69 MB/core HBM traffic (fp32 was 137).
  * One merged KV tile per batch, loaded by two 1 MB DMAs: K^T half on the
    SP HWDGE ring, V half on the ACT ring (two physical rings; subtile deps
    let the score matmuls start as soon as the K half lands). Layout:
    [128, 0:4096]=K^T [(h,d), t], [128, 4096:8192]=V [tloc, (j,h,d)].
  * All attention math on PE as 128-contraction bf16 matmuls:
      scores: lhsT = K^T chunk [128=(h,d), 128=t] (stationary, FWL),
        rhs = Q2 [128, 2] block-column -> [128=t, (j,h)] PSUM, 32 chunks.
      exp: 2 strided ACT ops (per head) -> e bf16 + per-head denominator
        accumulation; softmax max-subtraction skipped (scores ~ N(0,1)).
      x: lhsT = V chunk [128=t, 128=(h,d)] (stationary, FWL), rhs =
        e[:, j, :] [128, 2] -> [128=(h,d), 2] accumulated over j in PSUM;
        off-head column is discarded at the end (free PE lanes).
    ~64 LDW+MM pairs per batch ~= 2.1 us, under the DMA rate.
  * Weights/q/consts ride 2 packed DMAs on the scalar ring (6 small DMAs
    took 26 us due to descriptor overhead and starved phase 0).
  * Q2 built by 2 masked ACT ops (per-partition scale AP) straight from
    the projection PSUM -- no DVE dependency before the first score MM.
  * New-token/normalization terms that only need phase 0 are emitted inside
    iteration 0 to shorten the end tail; projections and out-proj run in
    bf16 (host-simulated end-to-end rel err 4.4e-3 vs the 2e-2 gate).
"""

import numpy as np
import ml_dtypes
from contextlib import ExitStack

import concourse.bass as bass
import concourse.tile as tile
from concourse import bacc, mybir
from concourse.bass_utils import run_bass_kernel_spmd

F32 = mybir.dt.float32
BF16 = mybir.dt.bfloat16
AX = mybir.AxisListType
ALU = mybir.AluOpType
ACTF = mybir.ActivationFunctionType

B = 32          # batch
DM = 1024       # model dim
H = 16          # total heads
DK = 64         # head dim
T = 4096        # cache length
NCORES = 8
HPC = H // NCORES   # 2 heads per core
HD = HPC * DK       # 128 per-core head dims
NCH = DM // 128     # 8 contraction chunks for the projections
NJ = T // 128       # 32 t-chunks of 128

KV_BUFS = 7         # merged KV tile prefetch depth (16 KB/partition)

WP = 3 * NCH * HD + NCH * B          # bf16 pack free size: wq|wk|wv|qT
FP = DM + 16                         # fp32 pack free size: woT|cst
# cst columns: 0=bq 1=bk 2=bv 3:11=bo/8 11=bq*maskA 12=bq*maskB 13=maskA 14=maskB


def _build_nc(repeat=1, variant="full"):
    # variant: "full" | "dma" (K/V loads only) | "nope" (no V matmuls)
    nc = bacc.Bacc(
        "TRN2",
        target_bir_lowering=False,
        debug=False,
        enable_asserts=False,
        num_devices=NCORES,
    )

    wpk = nc.dram_tensor("wpk", [128, WP], BF16, kind="ExternalInput").ap()
    fpk = nc.dram_tensor("fpk", [128, FP], F32, kind="ExternalInput").ap()
    # merged KV: [b, 128, 0:T]=K^T [(h,d), t] ; [b, 128, T:2T]=V [tloc, (j,h,d)]
    kvc = nc.dram_tensor("kvc", [B, 128, 2 * T], BF16, kind="ExternalInput").ap()
    outT = nc.dram_tensor("outT", [128, NCH * B], F32, kind="ExternalOutput").ap()

    with ExitStack() as ctx:
        tc = ctx.enter_context(tile.TileContext(nc))
        const = ctx.enter_context(tc.tile_pool(name="const", bufs=1))
        psum = ctx.enter_context(tc.tile_pool(name="psum", bufs=1, space="PSUM"))

        # ---- packed constants into SBUF (2 DMAs, scalar ring) ----
        wpk_sb = const.tile([128, WP], BF16, tag="wpk")
        fpk_sb = const.tile([128, FP], F32, tag="fpk")
        nc.scalar.dma_start(wpk_sb[:], wpk)
        nc.scalar.dma_start(fpk_sb[:], fpk)

        wq_sb = wpk_sb[:, 0:1024].rearrange("p (c f) -> p c f", c=NCH)
        wk_sb = wpk_sb[:, 1024:2048].rearrange("p (c f) -> p c f", c=NCH)
        wv_sb = wpk_sb[:, 2048:3072].rearrange("p (c f) -> p c f", c=NCH)
        qT_sb = wpk_sb[:, 3072:3072 + NCH * B].rearrange("p (c f) -> p c f", c=NCH)
        wo_sb = fpk_sb[:, 0:DM]
        cst_sb = fpk_sb[:, DM:DM + 16]

        ones_sb = const.tile([128, 1], F32, tag="ones")
        onerow_sb = const.tile([1, 64], F32, tag="onerow")
        nc.vector.memset(ones_sb[:], 1.0)
        nc.vector.memset(onerow_sb[:], 1.0)

        dpartA = const.tile([128, B], F32, tag="dpA")  # head-0 denom partials
        dpartB = const.tile([128, B], F32, tag="dpB")  # head-1 denom partials

        # ---- phase 0: projections Q^T, Knew^T, Vnew^T  [128=(h,d), B] ----
        QTp = psum.tile([128, B], F32, tag="p0")
        KTp = psum.tile([128, B], F32, tag="p1")
        VTp = psum.tile([128, B], F32, tag="p2")
        for c in range(NCH):
            st, sp = (c == 0), (c == NCH - 1)
            nc.tensor.matmul(QTp[:], wq_sb[:, c, :], qT_sb[:, c, :], start=st, stop=sp)
        for c in range(NCH):
            st, sp = (c == 0), (c == NCH - 1)
            nc.tensor.matmul(KTp[:], wk_sb[:, c, :], qT_sb[:, c, :], start=st, stop=sp)
        for c in range(NCH):
            st, sp = (c == 0), (c == NCH - 1)
            nc.tensor.matmul(VTp[:], wv_sb[:, c, :], qT_sb[:, c, :], start=st, stop=sp)

        # Q2 block-columns [128, B, 2] bf16 via masked ACTs (no DVE in the gate)
        Q2_sb = const.tile([128, B, HPC], BF16, tag="Q2")
        nc.scalar.activation(Q2_sb[:, :, 0], QTp[:], ACTF.Identity,
                             bias=cst_sb[:, 11:12], scale=cst_sb[:, 13:14])
        nc.scalar.activation(Q2_sb[:, :, 1], QTp[:], ACTF.Identity,
                             bias=cst_sb[:, 12:13], scale=cst_sb[:, 14:15])

        QT_sb = const.tile([128, B], F32, tag="QT")
        KnT_sb = const.tile([128, B], F32, tag="KnT")
        VnT_sb = const.tile([128, B], F32, tag="VnT")
        nc.scalar.activation(QT_sb[:], QTp[:], ACTF.Identity, bias=cst_sb[:, 0:1], scale=1.0)
        nc.scalar.activation(KnT_sb[:], KTp[:], ACTF.Identity, bias=cst_sb[:, 1:2], scale=1.0)
        nc.scalar.activation(VnT_sb[:], VTp[:], ACTF.Identity, bias=cst_sb[:, 2:3], scale=1.0)

        # ---- main loop over batches ----
        small = ctx.enter_context(tc.tile_pool(name="small", bufs=1))
        kpool = ctx.enter_context(tc.tile_pool(name="kp", bufs=KV_BUFS))
        epool = ctx.enter_context(tc.tile_pool(name="ep", bufs=3))
        spool = ctx.enter_context(tc.tile_pool(name="sp", bufs=2, space="PSUM"))

        xpsum = psum.tile([128, B, HPC], F32, tag="px")

        prev = None  # (b, vt, e) pending V-matmuls (software pipelining)

        def emit_v(pb, pvt, pe):
            for j in range(NJ):
                nc.tensor.matmul(
                    xpsum[:, pb, :], pvt[:, j], pe[:, j, :],
                    start=(j == 0), stop=(j == NJ - 1),
                )

        batches = [bb for _ in range(repeat) for bb in range(B)]
        for i, b in enumerate(batches):
            kvt = kpool.tile([128, 2 * T], BF16, tag="kv")
            # K via the SP HWDGE ring; V via SWDGE on the otherwise-idle
            # GpSimd engine. Keeps the ACT queue for exp only (it was the
            # latent bottleneck at ~3.8 us/batch with V-DMA issues + 4-way
            # exp: over the ~2.9 us/batch ring pace of faster parts).
            nc.sync.dma_start(kvt[:, 0:T], kvc[b, :, 0:T])
            nc.gpsimd.dma_start(kvt[:, T:2 * T], kvc[b, :, T:2 * T])
            kt = kvt[:, 0:T]
            vt = kvt[:, T:2 * T].rearrange("p (j f) -> p j f", j=NJ)

            if variant == "dma":
                scr0 = epool.tile([128, NJ, HPC], BF16, tag="e")
                nc.vector.tensor_copy(scr0[:, 0, :], kt[:, 0:2])
                nc.vector.tensor_copy(scr0[:, 1, :], vt[:, 0, 0:2])
                continue

            # scores: 32 chunk matmuls -> sp [128=t, (j, h)]
            sp = spool.tile([128, NJ, HPC], F32, tag="s")
            for j in range(NJ):
                nc.tensor.matmul(
                    sp[:, j, :], kt[:, j * 128:(j + 1) * 128], Q2_sb[:, b, :],
                    start=True, stop=True,
                )

            # exp (scale 1/sqrt(dk)) + per-head denominator partials
            e = epool.tile([128, NJ, HPC], BF16, tag="e")
            nc.scalar.activation(
                e[:, :, 0], sp[:, :, 0], ACTF.Exp, scale=0.125,
                accum_out=dpartA[:, b:b + 1],
            )
            nc.scalar.activation(
                e[:, :, 1], sp[:, :, 1], ACTF.Exp, scale=0.125,
                accum_out=dpartB[:, b:b + 1],
            )

            if i == 0 and variant == "full":
                # new-token terms (phase-0-only deps; emitted here so the PE
                # hits S[0] first, and the end tail stays short)
                prod2 = small.tile([128, B], F32, tag="prod2")
                nc.vector.tensor_mul(prod2[:], QT_sb[:], KnT_sb[:])
                snpA = psum.tile([1, B], F32, tag="p0")
                snpB = psum.tile([1, B], F32, tag="p1")
                nc.tensor.matmul(snpA[0:1, :], ones_sb[0:64, 0:1], prod2[0:64, :],
                                 start=True, stop=True, tile_position=(0, 0))
                nc.tensor.matmul(snpB[0:1, :], ones_sb[64:128, 0:1], prod2[64:128, :],
                                 start=True, stop=True, tile_position=(64, 0))
                e_new = small.tile([1, 2 * B], F32, tag="enew")
                nc.scalar.activation(e_new[0:1, 0:B], snpA[0:1, :], ACTF.Exp, scale=0.125)
                nc.scalar.activation(e_new[0:1, B:2 * B], snpB[0:1, :], ACTF.Exp, scale=0.125)
                erp = psum.tile([128, B], F32, tag="p2")
                nc.tensor.matmul(erp[0:64, :], onerow_sb[0:1, 0:64], e_new[0:1, 0:B],
                                 start=True, stop=True, tile_position=(0, 0))
                nc.tensor.matmul(erp[64:128, :], onerow_sb[0:1, 0:64], e_new[0:1, B:2 * B],
                                 start=True, stop=True, tile_position=(0, 64))
                tmp = small.tile([128, B], F32, tag="tmp")
                nc.vector.tensor_mul(tmp[:], VnT_sb[:], erp[:])
                # bf16 copy of wo for the out-proj, while DVE is idle
                wob_sb = small.tile([HD, DM], BF16, tag="wob")
                nc.vector.tensor_copy(wob_sb[:], wo_sb)

            if variant == "nope":
                continue

            if prev is not None:
                emit_v(*prev)
            prev = (b, vt, e)

        if variant == "full" and prev is not None:
            emit_v(*prev)

        # ---- epilogue ----
        if variant != "full":
            junk = small.tile([128, NCH * B], F32, tag="out")
            nc.vector.tensor_copy(junk[:], fpk_sb[:, 0:128].unsqueeze(1).broadcast_to([128, 2, 128]))
            nc.sync.dma_start(outT, junk[:])

        if variant == "full":
            # x diagonal extraction fused with the new-token fold
            xu = small.tile([128, B], F32, tag="xu")
            nc.vector.tensor_add(xu[0:64, :], xpsum[0:64, :, 0], tmp[0:64, :])
            nc.vector.tensor_add(xu[64:128, :], xpsum[64:128, :, 1], tmp[64:128, :])

            # denominators: full-partition sums of dpartA/dpartB + e_new
            dnpA = psum.tile([1, B], F32, tag="p0")
            dnpB = psum.tile([1, B], F32, tag="p1")
            nc.tensor.matmul(dnpA[0:1, :], ones_sb[:, 0:1], dpartA[:],
                             start=True, stop=True)
            nc.tensor.matmul(dnpB[0:1, :], ones_sb[:, 0:1], dpartB[:],
                             start=True, stop=True)
            dtot = small.tile([1, 2 * B], F32, tag="dtot")
            nc.vector.tensor_add(dtot[0:1, 0:B], dnpA[0:1, :], e_new[0:1, 0:B])
            nc.vector.tensor_add(dtot[0:1, B:2 * B], dnpB[0:1, :], e_new[0:1, B:2 * B])
            rcp = small.tile([1, 2 * B], F32, tag="rcp")
            nc.vector.reciprocal(rcp[0:1, :], dtot[0:1, :])

            rcpp = spool.tile([128, B], F32, tag="s")
            nc.tensor.matmul(rcpp[0:64, :], onerow_sb[0:1, 0:64], rcp[0:1, 0:B],
                             start=True, stop=True, tile_position=(0, 0))
            nc.tensor.matmul(rcpp[64:128, :], onerow_sb[0:1, 0:64], rcp[0:1, B:2 * B],
                             start=True, stop=True, tile_position=(0, 64))
            xs = small.tile([128, B], BF16, tag="xs")
            nc.vector.tensor_mul(xs[:], xu[:], rcpp[:])

            # output projection (bf16): out^T chunks [128, B] = woT-chunk.T @ x
            # (+ bo/8). spool ping-pong keeps PE-writes off the ACT-read bank.
            outsb = small.tile([128, NCH * B], F32, tag="out")
            for m in range(NCH):
                op = spool.tile([128, B], F32, tag="s")
                nc.tensor.matmul(op[:], wob_sb[:, m * 128:(m + 1) * 128], xs[:],
                                 start=True, stop=True)
                nc.scalar.activation(outsb[:, m * B:(m + 1) * B], op[:],
                                     ACTF.Identity, bias=cst_sb[:, 3 + m:4 + m], scale=1.0)
            nc.sync.dma_start(outT, outsb[:])

    nc.compile()
    return nc


_NC_CACHE = None


def _get_nc():
    global _NC_CACHE
    if _NC_CACHE is None:
        _NC_CACHE = _build_nc()
    return _NC_CACHE


def make_in_maps(q, key_pre, value_pre, wq, bq, wk, bk, wv, bv, wo, bo):
    bf = ml_dtypes.bfloat16
    q = np.asarray(q, np.float32)
    key_pre = np.asarray(key_pre, np.float32)
    value_pre = np.asarray(value_pre, np.float32)
    wq, bq = np.asarray(wq, np.float32), np.asarray(bq, np.float32)
    wk, bk = np.asarray(wk, np.float32), np.asarray(bk, np.float32)
    wv, bv = np.asarray(wv, np.float32), np.asarray(bv, np.float32)
    wo, bo = np.asarray(wo, np.float32), np.asarray(bo, np.float32)

    q2 = q.reshape(B, DM)
    qT8 = np.ascontiguousarray(q2.T.reshape(NCH, 128, B).transpose(1, 0, 2))
    bo8 = (bo / NCORES).reshape(NCH, 128).T  # [128, 8]
    maskA = np.zeros(128, np.float32)
    maskA[0:64] = 1.0
    maskB = 1.0 - maskA

    in_maps = []
    for c in range(NCORES):
        hs = slice(c * HD, (c + 1) * HD)
        heads = slice(c * HPC, (c + 1) * HPC)
        cstv = np.zeros((128, 16), np.float32)
        cstv[:, 0] = bq[hs]
        cstv[:, 1] = bk[hs]
        cstv[:, 2] = bv[hs]
        cstv[:, 3:11] = bo8
        cstv[:, 11] = bq[hs] * maskA
        cstv[:, 12] = bq[hs] * maskB
        cstv[:, 13] = maskA
        cstv[:, 14] = maskB
        wq8 = wq[hs].T.reshape(NCH, 128, HD).transpose(1, 0, 2).reshape(128, NCH * HD)
        wk8 = wk[hs].T.reshape(NCH, 128, HD).transpose(1, 0, 2).reshape(128, NCH * HD)
        wv8 = wv[hs].T.reshape(NCH, 128, HD).transpose(1, 0, 2).reshape(128, NCH * HD)
        wpack = np.concatenate(
            [wq8, wk8, wv8, qT8.reshape(128, NCH * B)], axis=1).astype(bf)
        fpack = np.concatenate(
            [np.ascontiguousarray(wo[:, hs].T), cstv], axis=1).astype(np.float32)
        # K^T: [B, 2, T, DK] -> [B, 2, DK, T] -> [B, 128, T]
        kh = key_pre[:, heads].transpose(0, 1, 3, 2).reshape(B, HD, T).astype(bf)
        # V: [B, 2, T, DK] -> [B, 2, NJ, 128, DK] -> [B, 128, NJ, 2, DK]
        vh = value_pre[:, heads].reshape(B, HPC, NJ, 128, DK) \
            .transpose(0, 3, 2, 1, 4).reshape(B, 128, T).astype(bf)
        kv = np.concatenate([kh, vh], axis=2)
        in_maps.append({
            "wpk": np.ascontiguousarray(wpack),
            "fpk": np.ascontiguousarray(fpack),
            "kvc": np.ascontiguousarray(kv),
        })
    return in_maps


def gather_output(results):
    total = np.zeros((B, DM), np.float64)
    for c in range(NCORES):
        r = results[c]["outT"]  # [128, NCH*B]
        x = r.reshape(128, NCH, B).transpose(2, 1, 0).reshape(B, DM)
        total += x
    return total.astype(np.float32).reshape(B, 1, DM)


def run(in_maps, trace=False, **kw):
    nc = _get_nc()
    return run_bass_kernel_spmd(nc, in_maps, core_ids=list(range(NCORES)),
                                trace=trace, **kw)


def kernel(q, key_pre, value_pre, wq, bq, wk, bk, wv, bv, wo, bo):
    in_maps = make_in_maps(q, key_pre, value_pre, wq, bq, wk, bk, wv, bv, wo, bo)
    res = run(in_maps, trace=False)
    return gather_output(res.results)


# revision 19
# speedup vs baseline: 1.1333x; 1.1333x over previous
"""Trainium2 Bass kernel: caching self multi-headed attention (decode step).

Problem: B=32, QLEN=1, DM=1024, H=16, DK=64, TCACHE=4096, fp32 in/out.
  out = MHA(q; KV cache) with QKV projections, cache append, softmax, out-proj.

Sharding (8 NeuronCores): tensor-parallel over heads. Core c owns heads
[2c, 2c+1]: column-parallel wq/wk/wv (128 output dims per core), KV cache
shards on the head dim, row-parallel wo giving a partial [32, 1024] output
per core; the host sums the 8 partials.

Design (final), from HW traces (v1 fp32: DMA 88%@314 GB/s, PE 91%; bf16
merged-DMA: stream runs ~425 GB/s; measured exec 206-208 us matches the
chip-wide HBM floor: 8 cores x 68.5 MB / ~2.95 TB/s + ~17 us preamble/tail;
a pure-DMA variant with no compute measures the same, so compute is fully
hidden):
  * KV cache bf16 on host -> 69 MB/core HBM traffic (fp32 was 137).
  * One merged KV tile per batch, loaded by two 1 MB DMAs: K^T half on the
    SP HWDGE ring, V half on the ACT ring (two physical rings; subtile deps
    let the score matmuls start as soon as the K half lands). Layout:
    [128, 0:4096]=K^T [(h,d), t], [128, 4096:8192]=V [tloc, (j,h,d)].
  * All attention math on PE as 128-contraction bf16 matmuls:
      scores: lhsT = K^T chunk [128=(h,d), 128=t] (stationary, FWL),
        rhs = Q2 [128, 2] block-column -> [128=t, (j,h)] PSUM, 32 chunks.
      exp: 2 strided ACT ops (per head) -> e bf16 + per-head denominator
        accumulation; softmax max-subtraction skipped (scores ~ N(0,1)).
      x: lhsT = V chunk [128=t, 128=(h,d)] (stationary, FWL), rhs =
        e[:, j, :] [128, 2] -> [128=(h,d), 2] accumulated over j in PSUM;
        off-head column is discarded at the end (free PE lanes).
    ~64 LDW+MM pairs per batch ~= 2.1 us, under the DMA rate.
  * Weights/q/consts ride 2 packed DMAs on the scalar ring (6 small DMAs
    took 26 us due to descriptor overhead and starved phase 0).
  * Q2 built by 2 masked ACT ops (per-partition scale AP) straight from
    the projection PSUM -- no DVE dependency before the first score MM.
  * New-token/normalization terms that only need phase 0 are emitted inside
    iteration 0 to shorten the end tail; projections and out-proj run in
    bf16 (host-simulated end-to-end rel err 4.4e-3 vs the 2e-2 gate).
"""

import numpy as np
import ml_dtypes
from contextlib import ExitStack

import concourse.bass as bass
import concourse.tile as tile
from concourse import bacc, mybir
from concourse.bass_utils import run_bass_kernel_spmd

F32 = mybir.dt.float32
BF16 = mybir.dt.bfloat16
AX = mybir.AxisListType
ALU = mybir.AluOpType
ACTF = mybir.ActivationFunctionType

B = 32          # batch
DM = 1024       # model dim
H = 16          # total heads
DK = 64         # head dim
T = 4096        # cache length
NCORES = 8
HPC = H // NCORES   # 2 heads per core
HD = HPC * DK       # 128 per-core head dims
NCH = DM // 128     # 8 contraction chunks for the projections
NJ = T // 128       # 32 t-chunks of 128

KV_BUFS = 7         # merged KV tile prefetch depth (16 KB/partition)

WP = 3 * NCH * HD + NCH * B          # bf16 pack free size: wq|wk|wv|qT
FP = DM + 16                         # fp32 pack free size: woT|cst
# cst columns: 0=bq 1=bk 2=bv 3:11=bo/8 11=bq*maskA 12=bq*maskB 13=maskA 14=maskB


def _build_nc(repeat=1, variant="full"):
    # variant: "full" | "dma" (K/V loads only) | "nope" (no V matmuls)
    nc = bacc.Bacc(
        "TRN2",
        target_bir_lowering=False,
        debug=False,
        enable_asserts=False,
        num_devices=NCORES,
    )

    wpk = nc.dram_tensor("wpk", [128, WP], BF16, kind="ExternalInput").ap()
    fpk = nc.dram_tensor("fpk", [128, FP], F32, kind="ExternalInput").ap()
    # merged KV: [b, 128, 0:T]=K^T [(h,d), t] ; [b, 128, T:2T]=V [tloc, (j,h,d)]
    kvc = nc.dram_tensor("kvc", [B, 128, 2 * T], BF16, kind="ExternalInput").ap()
    outT = nc.dram_tensor("outT", [128, NCH * B], F32, kind="ExternalOutput").ap()

    with ExitStack() as ctx:
        tc = ctx.enter_context(tile.TileContext(nc))
        const = ctx.enter_context(tc.tile_pool(name="const", bufs=1))
        psum = ctx.enter_context(tc.tile_pool(name="psum", bufs=1, space="PSUM"))

        # ---- packed constants into SBUF (2 DMAs, scalar ring) ----
        wpk_sb = const.tile([128, WP], BF16, tag="wpk")
        fpk_sb = const.tile([128, FP], F32, tag="fpk")
        nc.scalar.dma_start(wpk_sb[:], wpk)
        nc.scalar.dma_start(fpk_sb[:], fpk)

        wq_sb = wpk_sb[:, 0:1024].rearrange("p (c f) -> p c f", c=NCH)
        wk_sb = wpk_sb[:, 1024:2048].rearrange("p (c f) -> p c f", c=NCH)
        wv_sb = wpk_sb[:, 2048:3072].rearrange("p (c f) -> p c f", c=NCH)
        qT_sb = wpk_sb[:, 3072:3072 + NCH * B].rearrange("p (c f) -> p c f", c=NCH)
        wo_sb = fpk_sb[:, 0:DM]
        cst_sb = fpk_sb[:, DM:DM + 16]

        ones_sb = const.tile([128, 1], F32, tag="ones")
        onerow_sb = const.tile([1, 64], F32, tag="onerow")
        nc.vector.memset(ones_sb[:], 1.0)
        nc.vector.memset(onerow_sb[:], 1.0)

        dpartA = const.tile([128, B], F32, tag="dpA")  # head-0 denom partials
        dpartB = const.tile([128, B], F32, tag="dpB")  # head-1 denom partials

        # ---- phase 0: projections Q^T, Knew^T, Vnew^T  [128=(h,d), B] ----
        QTp = psum.tile([128, B], F32, tag="p0")
        KTp = psum.tile([128, B], F32, tag="p1")
        VTp = psum.tile([128, B], F32, tag="p2")
        for c in range(NCH):
            st, sp = (c == 0), (c == NCH - 1)
            nc.tensor.matmul(QTp[:], wq_sb[:, c, :], qT_sb[:, c, :], start=st, stop=sp)
        for c in range(NCH):
            st, sp = (c == 0), (c == NCH - 1)
            nc.tensor.matmul(KTp[:], wk_sb[:, c, :], qT_sb[:, c, :], start=st, stop=sp)
        for c in range(NCH):
            st, sp = (c == 0), (c == NCH - 1)
            nc.tensor.matmul(VTp[:], wv_sb[:, c, :], qT_sb[:, c, :], start=st, stop=sp)

        # Q2 block-columns [128, B, 2] bf16 via masked ACTs (no DVE in the gate)
        Q2_sb = const.tile([128, B, HPC], BF16, tag="Q2")
        nc.scalar.activation(Q2_sb[:, :, 0], QTp[:], ACTF.Identity,
                             bias=cst_sb[:, 11:12], scale=cst_sb[:, 13:14])
        nc.scalar.activation(Q2_sb[:, :, 1], QTp[:], ACTF.Identity,
                             bias=cst_sb[:, 12:13], scale=cst_sb[:, 14:15])

        QT_sb = const.tile([128, B], F32, tag="QT")
        KnT_sb = const.tile([128, B], F32, tag="KnT")
        VnT_sb = const.tile([128, B], F32, tag="VnT")
        nc.scalar.activation(QT_sb[:], QTp[:], ACTF.Identity, bias=cst_sb[:, 0:1], scale=1.0)
        nc.scalar.activation(KnT_sb[:], KTp[:], ACTF.Identity, bias=cst_sb[:, 1:2], scale=1.0)
        nc.scalar.activation(VnT_sb[:], VTp[:], ACTF.Identity, bias=cst_sb[:, 2:3], scale=1.0)

        # ---- main loop over batches ----
        small = ctx.enter_context(tc.tile_pool(name="small", bufs=1))
        kpool = ctx.enter_context(tc.tile_pool(name="kp", bufs=KV_BUFS))
        epool = ctx.enter_context(tc.tile_pool(name="ep", bufs=3))
        spool = ctx.enter_context(tc.tile_pool(name="sp", bufs=2, space="PSUM"))

        xpsum = psum.tile([128, B, HPC], F32, tag="px")

        prev = None  # (b, vt, e) pending V-matmuls (software pipelining)

        def emit_v(pb, pvt, pe):
            for j in range(NJ):
                nc.tensor.matmul(
                    xpsum[:, pb, :], pvt[:, j], pe[:, j, :],
                    start=(j == 0), stop=(j == NJ - 1),
                )

        batches = [bb for _ in range(repeat) for bb in range(B)]
        for i, b in enumerate(batches):
            kvt = kpool.tile([128, 2 * T], BF16, tag="kv")
            # K on the SP HWDGE ring, V on the ACT HWDGE ring, one 1 MB
            # transfer each (SWDGE measured ~25 us slower for the V stream;
            # 4 split DMAs + 4-way exp overloaded the ACT queue to ~3.8
            # us/batch -- this keeps it at ~2.0, under the ~2.9 us/batch
            # ring pace of faster parts).
            nc.sync.dma_start(kvt[:, 0:T], kvc[b, :, 0:T])
            nc.scalar.dma_start(kvt[:, T:2 * T], kvc[b, :, T:2 * T])
            kt = kvt[:, 0:T]
            vt = kvt[:, T:2 * T].rearrange("p (j f) -> p j f", j=NJ)

            if variant == "dma":
                scr0 = epool.tile([128, NJ, HPC], BF16, tag="e")
                nc.vector.tensor_copy(scr0[:, 0, :], kt[:, 0:2])
                nc.vector.tensor_copy(scr0[:, 1, :], vt[:, 0, 0:2])
                continue

            # scores: 32 chunk matmuls -> sp [128=t, (j, h)]
            sp = spool.tile([128, NJ, HPC], F32, tag="s")
            for j in range(NJ):
                nc.tensor.matmul(
                    sp[:, j, :], kt[:, j * 128:(j + 1) * 128], Q2_sb[:, b, :],
                    start=True, stop=True,
                )

            # exp (scale 1/sqrt(dk)) + per-head denominator partials
            e = epool.tile([128, NJ, HPC], BF16, tag="e")
            nc.scalar.activation(
                e[:, :, 0], sp[:, :, 0], ACTF.Exp, scale=0.125,
                accum_out=dpartA[:, b:b + 1],
            )
            nc.scalar.activation(
                e[:, :, 1], sp[:, :, 1], ACTF.Exp, scale=0.125,
                accum_out=dpartB[:, b:b + 1],
            )

            if i == 0 and variant == "full":
                # new-token terms (phase-0-only deps; emitted here so the PE
                # hits S[0] first, and the end tail stays short)
                prod2 = small.tile([128, B], F32, tag="prod2")
                nc.vector.tensor_mul(prod2[:], QT_sb[:], KnT_sb[:])
                snpA = psum.tile([1, B], F32, tag="p0")
                snpB = psum.tile([1, B], F32, tag="p1")
                nc.tensor.matmul(snpA[0:1, :], ones_sb[0:64, 0:1], prod2[0:64, :],
                                 start=True, stop=True, tile_position=(0, 0))
                nc.tensor.matmul(snpB[0:1, :], ones_sb[64:128, 0:1], prod2[64:128, :],
                                 start=True, stop=True, tile_position=(64, 0))
                e_new = small.tile([1, 2 * B], F32, tag="enew")
                nc.scalar.activation(e_new[0:1, 0:B], snpA[0:1, :], ACTF.Exp, scale=0.125)
                nc.scalar.activation(e_new[0:1, B:2 * B], snpB[0:1, :], ACTF.Exp, scale=0.125)
                erp = psum.tile([128, B], F32, tag="p2")
                nc.tensor.matmul(erp[0:64, :], onerow_sb[0:1, 0:64], e_new[0:1, 0:B],
                                 start=True, stop=True, tile_position=(0, 0))
                nc.tensor.matmul(erp[64:128, :], onerow_sb[0:1, 0:64], e_new[0:1, B:2 * B],
                                 start=True, stop=True, tile_position=(0, 64))
                tmp = small.tile([128, B], F32, tag="tmp")
                nc.vector.tensor_mul(tmp[:], VnT_sb[:], erp[:])
                # bf16 copy of wo for the out-proj, while DVE is idle
                wob_sb = small.tile([HD, DM], BF16, tag="wob")
                nc.vector.tensor_copy(wob_sb[:], wo_sb)

            if variant == "nope":
                continue

            if prev is not None:
                emit_v(*prev)
            prev = (b, vt, e)

        if variant == "full" and prev is not None:
            emit_v(*prev)

        # ---- epilogue ----
        if variant != "full":
            junk = small.tile([128, NCH * B], F32, tag="out")
            nc.vector.tensor_copy(junk[:], fpk_sb[:, 0:128].unsqueeze(1).broadcast_to([128, 2, 128]))
            nc.sync.dma_start(outT, junk[:])

        if variant == "full":
            # x diagonal extraction fused with the new-token fold
            xu = small.tile([128, B], F32, tag="xu")
            nc.vector.tensor_add(xu[0:64, :], xpsum[0:64, :, 0], tmp[0:64, :])
            nc.vector.tensor_add(xu[64:128, :], xpsum[64:128, :, 1], tmp[64:128, :])

            # denominators: full-partition sums of dpartA/dpartB + e_new
            dnpA = psum.tile([1, B], F32, tag="p0")
            dnpB = psum.tile([1, B], F32, tag="p1")
            nc.tensor.matmul(dnpA[0:1, :], ones_sb[:, 0:1], dpartA[:],
                             start=True, stop=True)
            nc.tensor.matmul(dnpB[0:1, :], ones_sb[:, 0:1], dpartB[:],
                             start=True, stop=True)
            dtot = small.tile([1, 2 * B], F32, tag="dtot")
            nc.vector.tensor_add(dtot[0:1, 0:B], dnpA[0:1, :], e_new[0:1, 0:B])
            nc.vector.tensor_add(dtot[0:1, B:2 * B], dnpB[0:1, :], e_new[0:1, B:2 * B])
            rcp = small.tile([1, 2 * B], F32, tag="rcp")
            nc.vector.reciprocal(rcp[0:1, :], dtot[0:1, :])

            rcpp = spool.tile([128, B], F32, tag="s")
            nc.tensor.matmul(rcpp[0:64, :], onerow_sb[0:1, 0:64], rcp[0:1, 0:B],
                             start=True, stop=True, tile_position=(0, 0))
            nc.tensor.matmul(rcpp[64:128, :], onerow_sb[0:1, 0:64], rcp[0:1, B:2 * B],
                             start=True, stop=True, tile_position=(0, 64))
            xs = small.tile([128, B], BF16, tag="xs")
            nc.vector.tensor_mul(xs[:], xu[:], rcpp[:])

            # output projection (bf16): out^T chunks [128, B] = woT-chunk.T @ x
            # (+ bo/8). spool ping-pong keeps PE-writes off the ACT-read bank.
            outsb = small.tile([128, NCH * B], F32, tag="out")
            for m in range(NCH):
                op = spool.tile([128, B], F32, tag="s")
                nc.tensor.matmul(op[:], wob_sb[:, m * 128:(m + 1) * 128], xs[:],
                                 start=True, stop=True)
                nc.scalar.activation(outsb[:, m * B:(m + 1) * B], op[:],
                                     ACTF.Identity, bias=cst_sb[:, 3 + m:4 + m], scale=1.0)
            nc.sync.dma_start(outT, outsb[:])

    nc.compile()
    return nc


_NC_CACHE = None


def _get_nc():
    global _NC_CACHE
    if _NC_CACHE is None:
        _NC_CACHE = _build_nc()
    return _NC_CACHE


def make_in_maps(q, key_pre, value_pre, wq, bq, wk, bk, wv, bv, wo, bo):
    bf = ml_dtypes.bfloat16
    q = np.asarray(q, np.float32)
    key_pre = np.asarray(key_pre, np.float32)
    value_pre = np.asarray(value_pre, np.float32)
    wq, bq = np.asarray(wq, np.float32), np.asarray(bq, np.float32)
    wk, bk = np.asarray(wk, np.float32), np.asarray(bk, np.float32)
    wv, bv = np.asarray(wv, np.float32), np.asarray(bv, np.float32)
    wo, bo = np.asarray(wo, np.float32), np.asarray(bo, np.float32)

    q2 = q.reshape(B, DM)
    qT8 = np.ascontiguousarray(q2.T.reshape(NCH, 128, B).transpose(1, 0, 2))
    bo8 = (bo / NCORES).reshape(NCH, 128).T  # [128, 8]
    maskA = np.zeros(128, np.float32)
    maskA[0:64] = 1.0
    maskB = 1.0 - maskA

    in_maps = []
    for c in range(NCORES):
        hs = slice(c * HD, (c + 1) * HD)
        heads = slice(c * HPC, (c + 1) * HPC)
        cstv = np.zeros((128, 16), np.float32)
        cstv[:, 0] = bq[hs]
        cstv[:, 1] = bk[hs]
        cstv[:, 2] = bv[hs]
        cstv[:, 3:11] = bo8
        cstv[:, 11] = bq[hs] * maskA
        cstv[:, 12] = bq[hs] * maskB
        cstv[:, 13] = maskA
        cstv[:, 14] = maskB
        wq8 = wq[hs].T.reshape(NCH, 128, HD).transpose(1, 0, 2).reshape(128, NCH * HD)
        wk8 = wk[hs].T.reshape(NCH, 128, HD).transpose(1, 0, 2).reshape(128, NCH * HD)
        wv8 = wv[hs].T.reshape(NCH, 128, HD).transpose(1, 0, 2).reshape(128, NCH * HD)
        wpack = np.concatenate(
            [wq8, wk8, wv8, qT8.reshape(128, NCH * B)], axis=1).astype(bf)
        fpack = np.concatenate(
            [np.ascontiguousarray(wo[:, hs].T), cstv], axis=1).astype(np.float32)
        # K^T: [B, 2, T, DK] -> [B, 2, DK, T] -> [B, 128, T]
        kh = key_pre[:, heads].transpose(0, 1, 3, 2).reshape(B, HD, T).astype(bf)
        # V: [B, 2, T, DK] -> [B, 2, NJ, 128, DK] -> [B, 128, NJ, 2, DK]
        vh = value_pre[:, heads].reshape(B, HPC, NJ, 128, DK) \
            .transpose(0, 3, 2, 1, 4).reshape(B, 128, T).astype(bf)
        kv = np.concatenate([kh, vh], axis=2)
        in_maps.append({
            "wpk": np.ascontiguousarray(wpack),
            "fpk": np.ascontiguousarray(fpack),
            "kvc": np.ascontiguousarray(kv),
        })
    return in_maps


def gather_output(results):
    total = np.zeros((B, DM), np.float64)
    for c in range(NCORES):
        r = results[c]["outT"]  # [128, NCH*B]
        x = r.reshape(128, NCH, B).transpose(2, 1, 0).reshape(B, DM)
        total += x
    return total.astype(np.float32).reshape(B, 1, DM)


def run(in_maps, trace=False, **kw):
    nc = _get_nc()
    return run_bass_kernel_spmd(nc, in_maps, core_ids=list(range(NCORES)),
                                trace=trace, **kw)


def kernel(q, key_pre, value_pre, wq, bq, wk, bk, wv, bv, wo, bo):
    in_maps = make_in_maps(q, key_pre, value_pre, wq, bq, wk, bk, wv, bv, wo, bo)
    res = run(in_maps, trace=False)
    return gather_output(res.results)
